# revision 53
# baseline (speedup 1.0000x reference)
"""CrossMerge kernel for trn2 — v7 (fp16, PE broadcast, DVE chunk-folding).

Math (per batch element):
    means_i = mean over C of g_i              (4, H, W)
    logits  = w_proj @ means + b_proj         (4, H, W)
    w       = softmax(logits, axis=0)         (4, H, W)
    out     = sum_i g_i * w_i                 (C, H, W)

Sharding: data-parallel over batch B=8 across 8 cores; weights replicated;
no cross-device communication.

Measured history: v3 fp32 247us; v4 fp16 157.5us (PE-bound 148us busy);
v5b/v6 gpsimd-broadcast variants 182-186us — the gpsimd fp32 broadcast
writes 19-38MB through the SBUF ports and stalls concurrent DVE ops to
3.1us/op (vs their 952ns median), so broadcasts stay on PE+PSUM (PSUM has
its own ports; the ACT staging writes only the final 9.4MB of fp16).

Final (v16r) measured: ~139-142us (best 138.3; adjacent-run A/B beat v14
by ~1.6us).  Session-5 addition: grids 2,3's weight broadcast is routed
through the DMA engines instead of PE+ACT — W4 rows bounce through a
small Internal-DRAM scratch (DMA rejects stride-0 SBUF partition dims;
a flat DRAM row re-read 128x via [[0,128],[1,w]] is legal), costing PE
two fewer column-passes and ACT two fewer staging passes.  The broadcast
DMAs are issued BEFORE the 4.7MB dma_in prefetch so they don't queue
behind it (issuing them after measured +4us).
Previous checkpoint (v14): ~140.5us median (139.6-141.7 over 6 runs; occasional
~175us outliers under external device contention).  PE 92%-occupied at
135us busy — the engine-balance limit of this decomposition.
Session-3 additions on top of v7: tapered tile widths TW (small first
tile shortens the fill, small last tile + interleaved per-jslice drain
shortens the tail), and the final add+store deferred to finish_stage
after the next tile's narrow smalls so W4d(d) completes ~2us earlier and
the next iteration's PE broadcast never stalls on it.

v7 design:
 - Grids fp16 on HOST (HBM 23.7MB/core); output fp16, host upconverts.
 - DVE merge in fp16 2x_1p mode (0.52 ns/col, HW-verified): products as
   4 chunk-paired ops [128,2,1536] (wq broadcast over the chunk axis via
   an explicit 0-stride AP dim) + 3 paired adds, halving op count.
 - PE cost is passes x cols x 0.85ns (observed throttled clock) + 93ns
   LDWEIGHTS per matmul.  v4 ran 13 column passes on PE; v7 runs 10:
   grids 0-2's C-chunks are pre-folded on DVE (t_i = g_c0 + g_c1, fp16
   2x), so logits need 5 accumulating matmuls per jslice instead of 8.
   Folding all 4 would tip DVE past PE; 3 balances the two engines.
 - Broadcast staging via [128,768] PSUM tiles: per grid per 768-block,
   two matmuls (N=512+256, each within one PSUM bank) + ONE wide ACT
   copy PSUM->SBUF fp16 (halves v4's ACT op count and sem load).
 - Narrow tail per jslice: exp (ACT, scale=1/C exp-trick, bias=b_proj),
   S4 denominator (PE, ones lhsT), reciprocal_approx_fast (DVE, fp32),
   W4 = E*R4 -> fp16 into a d-tile-wide W4d (next iter's bcast rhs).
 - Per-iter emission (engine queue order is what matters):
     dma_in(d+1) | DVE folds(d) | PE bcast(d-1) + ACT staging | DVE
     products/adds(d-1) + dma_out(d-1) | narrow(d) | (DVE recip/W4 last)
   Folds go first on DVE so PE's logits(d) unblock early; products(d-1)
   keep DVE busy while the narrow(d) PE->ACT->PE chain round-trips.
 - The flush (last tile) runs broadcast+wide in two 768-col halves so the
   DVE wide work of half 1 overlaps the PE/ACT broadcast of half 2
   (-7us of serial drain).
 - Negative results (measured): all-gpsimd broadcast (fp32 partition_
   broadcast) 182-186us from SBUF-port DVE stalls; gpsimd partition_all_
   reduce for the softmax denominator 191us (3.5-3.9us/op + ~1us sem
   events on the Q7); fold-4 180us; 1024/512 bcast staging blocks 161us.
"""

import os
import sys

import numpy as np

try:
    import concourse.bass as bass
except ImportError:  # fresh grading dir: concourse lives in the container repo
    sys.path.insert(0, "/opt/trn_rl_repo")
    import concourse.bass as bass

from contextlib import ExitStack

import concourse.tile as tile
from concourse import bacc, mybir
from concourse.bass_utils import run_bass_kernel_spmd

B, C, H, W = 8, 256, 96, 96
HW = H * W  # 9216
NCORES = 8
CPB = C // 128  # 2 partition chunks per core
MAXW = 1536  # pool sizing; per-tile widths taper at both ends
# small first tile shortens the fill (dma+narrow before any wide work);
# small last tile shortens the drain (bcast+wide after the last narrow)
TW = [512, 1536, 1536, 1536, 1536, 1536, 1024]
assert sum(TW) == HW and all(w % 512 == 0 for w in TW)
ND = len(TW)
NFOLD = 3  # grids 0..NFOLD-1 chunk-folded on DVE for the logits pass
BLK = 768  # bcast staging block (2 matmuls + 1 ACT copy per grid)

F32 = mybir.dt.float32
F16 = mybir.dt.float16
AF = mybir.ActivationFunctionType

_CACHE = {}


def bcast2(ap, n):
    """AP broadcast over a new middle 'chunk' axis of size n (stride 0)."""
    return bass.AP(ap.tensor, ap.offset, [ap.ap[0], [0, n], ap.ap[-1]])


def build_program():
    nc = bacc.Bacc("TRN2", debug=False, num_devices=NCORES)

    gall_d = nc.dram_tensor("gall", [4, C, HW], F16, kind="ExternalInput").ap()
    # fp16 constants: cols 0-511 sel (one-hot bcast lhsT rows 0-3), cols
    # 512-527 ws (w[o,i] at col 512+4i+o, replicated down partitions),
    # cols 528-531 ones4x4
    ch_d = nc.dram_tensor("cblob16", [128, 532], F16, kind="ExternalInput").ap()
    # fp32 constants: col 0 rows 0-3 = b_proj
    cb_d = nc.dram_tensor("cblob", [128, 1], F32, kind="ExternalInput").ap()
    out = nc.dram_tensor("out", [C, HW], F16, kind="ExternalOutput").ap()
    # HBM bounce for the DMA-routed weight broadcast of grids 2,3: DMA
    # rejects stride-0 SBUF partition dims, but a flat DRAM source row can
    # be re-read 128x ([[0,128],[1,w]]), turning the broadcast into pure
    # DMA-engine work (no PE pass, no ACT staging).  Double-buffered by
    # d%2 (written end of narrow(d), read at iter d+1's bcast).
    wscr = nc.dram_tensor("wscr", [2, 2, MAXW], F16, kind="Internal").ap()

    with tile.TileContext(nc) as tc, ExitStack() as ctx:
        const = ctx.enter_context(tc.tile_pool(name="const", bufs=1))
        gin = ctx.enter_context(tc.tile_pool(name="gin", bufs=3))
        outp = ctx.enter_context(tc.tile_pool(name="outp", bufs=2))
        foldp = ctx.enter_context(tc.tile_pool(name="foldp", bufs=2))
        narrow = ctx.enter_context(tc.tile_pool(name="narrow", bufs=3))
        wqp = ctx.enter_context(tc.tile_pool(name="wqp", bufs=2))
        prod = ctx.enter_context(tc.tile_pool(name="prod", bufs=3))
        qpool = ctx.enter_context(tc.tile_pool(name="qpool", bufs=3))
        ps_nar = ctx.enter_context(tc.tile_pool(name="psnar", bufs=2, space="PSUM"))
        ps_wb = ctx.enter_context(tc.tile_pool(name="pswb", bufs=2, space="PSUM"))

        ch = const.tile([128, 532], F16)
        nc.sync.dma_start(out=ch[:], in_=ch_d)
        cb = const.tile([128, 1], F32)
        nc.sync.dma_start(out=cb[:], in_=cb_d)
        bv = cb[0:4, 0:1]

        def ws_i(i):  # [128, 4] logits lhsT for grid i
            return ch[:, 512 + 4 * i : 512 + 4 * i + 4]

        def sel_i(i):  # [4, 128] bcast lhsT for grid i
            return ch[0:4, 128 * i : 128 * (i + 1)]

        ones4 = ch[0:4, 528:532]

        # Warmup matmul: absorbs the const-blob DMA wait on the PE clock.
        warm = ps_nar.tile([128, 512], F32, tag="smx")
        nc.tensor.matmul(warm[0:4, 0:16], lhsT=ch[0:4, 0:4], rhs=ch[0:4, 0:16],
                         start=True, stop=True)

        def fold_stage(gat, w):
            """DVE chunk-fold for grids 0..NFOLD-1 (fp16 2x): the logits
            contraction over (grid, chunk) shrinks from 8 to 5 matmuls."""
            ts = []
            for i in range(NFOLD):
                t = foldp.tile([128, MAXW], F16, tag=f"t{i}")
                nc.vector.tensor_add(t[:, 0:w], gat[:, i, 0, 0:w],
                                     gat[:, i, 1, 0:w])
                ts.append(t)
            return ts

        def narrow_stage(d, gat, ts):
            """Softmax chain: logits (jslice pairs) -> exp -> S4 -> recip
            -> W4 (fp16, d-tile-wide for next iter's bcast rhs).  Matmul
            outputs at PSUM base partition 0 (ISA constraint)."""
            jslc = [(x0, 512) for x0 in range(0, TW[d], 512)]
            W4d = narrow.tile([4, MAXW], F16, tag="W4", bufs=2)
            for pair in [jslc[k : k + 2] for k in range(0, len(jslc), 2)]:
                Ls, Es = [], []
                for x0, n in pair:
                    L = ps_nar.tile([128, 512], F32, tag="smx")
                    Ls.append(L[0:4, 0:n])
                    rhss = [ts[i][:, x0 : x0 + n] for i in range(NFOLD)]
                    rhss += [
                        gat[:, i, c, x0 : x0 + n]
                        for i in range(NFOLD, 4)
                        for c in range(CPB)
                    ]
                    lhss = [ws_i(i) for i in range(NFOLD)] + [
                        ws_i(i) for i in range(NFOLD, 4) for _ in range(CPB)
                    ]
                    for k, (lh, rh) in enumerate(zip(lhss, rhss)):
                        nc.tensor.matmul(Ls[-1], lhsT=lh, rhs=rh,
                                         start=(k == 0),
                                         stop=(k == len(rhss) - 1))
                for pi, (x0, n) in enumerate(pair):
                    E = narrow.tile([4, 512], F16, tag="E")
                    nc.scalar.activation(E[0:4, 0:n], Ls[pi], AF.Exp,
                                         bias=bv, scale=1.0 / C)
                    Es.append(E[0:4, 0:n])
                S4s = []
                for pi, (x0, n) in enumerate(pair):
                    S4 = ps_nar.tile([4, 512], F32, tag="S4")
                    nc.tensor.matmul(S4[0:4, 0:n], lhsT=ones4, rhs=Es[pi],
                                     start=True, stop=True)
                    S4s.append(S4[0:4, 0:n])
                for pi, (x0, n) in enumerate(pair):
                    R4 = narrow.tile([4, 512], F32, tag="R4", bufs=2)
                    nc.vector.reciprocal_approx_fast(R4[0:4, 0:n], S4s[pi])
                    nc.vector.tensor_mul(W4d[0:4, x0 : x0 + n], Es[pi],
                                         R4[0:4, 0:n])
            nc.sync.dma_start(out=wscr[d % 2, :, 0 : TW[d]],
                              in_=W4d[2:4, 0 : TW[d]])
            return W4d

        def bcast_stage(prev):
            """PE broadcast of W4 rows to 128 partitions, staged to fp16
            SBUF by wide [128,768] ACT copies (each matmul writes within a
            single PSUM bank; the copy spans banks, reads are unrestricted)."""
            if prev is None:
                return None
            d, gat, ot, W4d = prev
            w = TW[d]
            wq = {}
            for i in range(4):
                wqt = wqp.tile([128, MAXW], F16, tag=f"wq{i}")
                wq[i] = wqt
            for i in (2, 3):
                row = wscr[d % 2, i - 2 : i - 1, 0:w]
                src = bass.AP(row.tensor, row.offset, [[0, 128], [1, w]])
                nc.sync.dma_start(out=wq[i][:, 0:w], in_=src)
            for b0 in range(0, w, BLK):
                bw = min(BLK, w - b0)
                for i in (0, 1):
                    Wb = ps_wb.tile([128, BLK], F32, tag="wb")
                    for s0 in range(0, bw, 512):
                        n = min(512, bw - s0)
                        nc.tensor.matmul(
                            Wb[:, s0 : s0 + n],
                            lhsT=sel_i(i),
                            rhs=W4d[0:4, b0 + s0 : b0 + s0 + n],
                            start=True, stop=True,
                        )
                    nc.scalar.copy(wq[i][:, b0 : b0 + bw], Wb[:, 0:bw])
            return (d, gat, ot, wq)

        def wide_stage(staged):
            """DVE products + first add-tree level (chunk-paired ops).
            The final add + store happen in finish_stage AFTER the next
            tile's narrow smalls, so W4d(d) completes ~2us earlier and the
            next iteration's PE broadcast doesn't stall on it."""
            if staged is None:
                return None
            d, gat, ot, wq = staged
            w = TW[d]
            p = {}
            # grids 2,3 first: their wq arrives early via the broadcast
            # DMAs (issued at iter top), while wq0/wq1 wait on PE+ACT
            for i in (2, 3, 0, 1):
                pt = prod.tile([128, CPB, MAXW], F16, tag="p")
                nc.vector.tensor_mul(pt[:, :, 0:w], gat[:, i, :, 0:w],
                                     bcast2(wq[i][:, 0:w], CPB))
                p[i] = pt
                if i == 3:
                    q23 = qpool.tile([128, CPB, MAXW], F16, tag="q")
                    nc.vector.tensor_add(q23[:, :, 0:w], p[2][:, :, 0:w],
                                         p[3][:, :, 0:w])
            q01 = qpool.tile([128, CPB, MAXW], F16, tag="q")
            nc.vector.tensor_add(q01[:, :, 0:w], p[0][:, :, 0:w],
                                 p[1][:, :, 0:w])
            return (d, ot, q01, q23)

        def finish_stage(pend):
            if pend is None:
                return
            d, ot, q01, q23 = pend
            w = TW[d]
            nc.vector.tensor_add(ot[:, :, 0:w], q01[:, :, 0:w],
                                 q23[:, :, 0:w])
            n0 = sum(TW[:d])
            nc.sync.dma_start(
                out=out[:, n0 : n0 + w].rearrange("(c p) n -> p c n", c=CPB),
                in_=ot[:, :, 0:w],
            )

        def dma_in(d):
            n0 = sum(TW[:d])
            w = TW[d]
            gat = gin.tile([128, 4, CPB, MAXW], F16, tag="gall")
            nc.sync.dma_start(
                out=gat[:, :, :, 0:w],
                in_=gall_d[:, :, n0 : n0 + w].rearrange(
                    "i (c p) n -> p i c n", c=CPB
                ),
            )
            return gat

        gats = {0: dma_in(0)}
        prev = None  # (d, gat, ot, W4d) awaiting bcast+wide
        for d in range(ND - 1):
            gat = gats.pop(d)
            ts = fold_stage(gat, TW[d])
            # bcast first: its (tiny) broadcast DMAs must hit the sync queue
            # ahead of the 4.7MB dma_in so wq2/wq3 aren't delayed behind it
            staged = bcast_stage(prev)
            if d + 1 < ND:
                gats[d + 1] = dma_in(d + 1)
            pend = wide_stage(staged)
            ot = outp.tile([128, CPB, MAXW], F16, tag="ot")
            W4d = narrow_stage(d, gat, ts)
            finish_stage(pend)
            prev = (d, gat, ot, W4d)
        # Last tile: single-jslice narrow chains with the drain interleaved
        # per slice.  Each slice's recip/W4 feeds its drain broadcast
        # immediately, so the PE/ACT drain work overlaps the other slice's
        # DVE work instead of serializing after the whole narrow chain.
        d = ND - 1
        gat = gats.pop(d)
        ts = fold_stage(gat, TW[d])
        staged = bcast_stage(prev)
        finish_stage(wide_stage(staged))
        ot = outp.tile([128, CPB, MAXW], F16, tag="ot")
        w = TW[d]
        jslc = [(x0, 512) for x0 in range(0, w, 512)]
        W4d = narrow.tile([4, MAXW], F16, tag="W4", bufs=2)
        chain = []
        for x0, n in jslc:
            L = ps_nar.tile([128, 512], F32, tag="smx")
            Lv = L[0:4, 0:n]
            rhss = [ts[i][:, x0 : x0 + n] for i in range(NFOLD)] + [
                gat[:, i, c, x0 : x0 + n]
                for i in range(NFOLD, 4)
                for c in range(CPB)
            ]
            lhss = [ws_i(i) for i in range(NFOLD)] + [
                ws_i(i) for i in range(NFOLD, 4) for _ in range(CPB)
            ]
            for k, (lh, rh) in enumerate(zip(lhss, rhss)):
                nc.tensor.matmul(Lv, lhsT=lh, rhs=rh, start=(k == 0),
                                 stop=(k == len(rhss) - 1))
            E = narrow.tile([4, 512], F16, tag="E")
            nc.scalar.activation(E[0:4, 0:n], Lv, AF.Exp, bias=bv,
                                 scale=1.0 / C)
            S4 = ps_nar.tile([4, 512], F32, tag="S4")
            nc.tensor.matmul(S4[0:4, 0:n], lhsT=ones4, rhs=E[0:4, 0:n],
                             start=True, stop=True)
            chain.append((x0, n, E, S4))
        for x0, n, E, S4 in chain:
            R4 = narrow.tile([4, 512], F32, tag="R4", bufs=2)
            nc.vector.reciprocal_approx_fast(R4[0:4, 0:n], S4[0:4, 0:n])
            nc.vector.tensor_mul(W4d[0:4, x0 : x0 + n], E[0:4, 0:n],
                                 R4[0:4, 0:n])
        dr = []
        for x0, n, E, S4 in chain:
            wqh = {}
            for i in range(4):
                wqt = wqp.tile([128, MAXW], F16, tag=f"wq{i}")
                wqh[i] = wqt
                Wb = ps_wb.tile([128, BLK], F32, tag="wb")
                nc.tensor.matmul(Wb[:, 0:n], lhsT=sel_i(i),
                                 rhs=W4d[0:4, x0 : x0 + n],
                                 start=True, stop=True)
                nc.scalar.copy(wqt[:, x0 : x0 + n], Wb[:, 0:n])
            dr.append((x0, n, wqh))
        for x0, n, wqh in dr:
            p = {}
            for i in range(4):
                pt = prod.tile([128, CPB, MAXW], F16, tag="p")
                nc.vector.tensor_mul(
                    pt[:, :, 0:n], gat[:, i, :, x0 : x0 + n],
                    bcast2(wqh[i][:, x0 : x0 + n], CPB),
                )
                p[i] = pt
                if i == 1:
                    q01 = qpool.tile([128, CPB, MAXW], F16, tag="q")
                    nc.vector.tensor_add(q01[:, :, 0:n], p[0][:, :, 0:n],
                                         p[1][:, :, 0:n])
            q23 = qpool.tile([128, CPB, MAXW], F16, tag="q")
            nc.vector.tensor_add(q23[:, :, 0:n], p[2][:, :, 0:n],
                                 p[3][:, :, 0:n])
            nc.vector.tensor_add(ot[:, :, x0 : x0 + n], q01[:, :, 0:n],
                                 q23[:, :, 0:n])
            n0 = sum(TW[:d]) + x0
            nc.sync.dma_start(
                out=out[:, n0 : n0 + n].rearrange("(c p) n -> p c n", c=CPB),
                in_=ot[:, :, x0 : x0 + n],
            )

    nc.compile()
    return nc


def _get_program():
    if "nc" not in _CACHE:
        _CACHE["nc"] = build_program()
    return _CACHE["nc"]


def make_cblobs(w_proj, b_proj):
    w = np.asarray(w_proj, dtype=np.float32)
    b = np.asarray(b_proj, dtype=np.float32)
    ch = np.zeros((128, 532), dtype=np.float16)
    sel = np.repeat(np.eye(4, dtype=np.float16), 128, axis=1)
    ch[0:4, 0:512] = sel
    for i in range(4):
        for o in range(4):
            ch[:, 512 + 4 * i + o] = np.float16(w[o, i])
    ch[0:4, 528:532] = 1.0
    cb = np.zeros((128, 1), dtype=np.float32)
    cb[0:4, 0] = b
    return ch, cb


LAST_RESULT = None


def kernel(g0, g1, g2, g3, w_proj, b_proj):
    global LAST_RESULT
    nc = _get_program()

    ch, cb = make_cblobs(w_proj, b_proj)

    gall = np.stack(
        [np.asarray(x).reshape(B, C, HW).astype(np.float16) for x in (g0, g1, g2, g3)],
        axis=1,
    )  # (B, 4, C, HW) fp16
    in_maps = []
    for bi in range(NCORES):
        m = {"gall": np.ascontiguousarray(gall[bi]), "cblob16": ch, "cblob": cb}
        in_maps.append(m)

    res = run_bass_kernel_spmd(
        nc,
        in_maps,
        list(range(NCORES)),
        trace=bool(int(os.environ.get("CM_TRACE", "0"))),
        tmpdir=os.environ.get("CM_TRACE_DIR") or None,
    )
    LAST_RESULT = res
    out_full = np.stack(
        [
            res.results[bi]["out"].astype(np.float32).reshape(C, H, W)
            for bi in range(NCORES)
        ],
        axis=0,
    )
    return out_full


# revision 54
# speedup vs baseline: 1.0647x; 1.0647x over previous
"""CrossMerge kernel for trn2 — v7 (fp16, PE broadcast, DVE chunk-folding).

Math (per batch element):
    means_i = mean over C of g_i              (4, H, W)
    logits  = w_proj @ means + b_proj         (4, H, W)
    w       = softmax(logits, axis=0)         (4, H, W)
    out     = sum_i g_i * w_i                 (C, H, W)

Sharding: data-parallel over batch B=8 across 8 cores; weights replicated;
no cross-device communication.

Measured history: v3 fp32 247us; v4 fp16 157.5us (PE-bound 148us busy);
v5b/v6 gpsimd-broadcast variants 182-186us — the gpsimd fp32 broadcast
writes 19-38MB through the SBUF ports and stalls concurrent DVE ops to
3.1us/op (vs their 952ns median), so broadcasts stay on PE+PSUM (PSUM has
its own ports; the ACT staging writes only the final 9.4MB of fp16).

Final (v16r) measured: ~139-142us (best 138.3; adjacent-run A/B beat v14
by ~1.6us).  Session-5 addition: grids 2,3's weight broadcast is routed
through the DMA engines instead of PE+ACT — W4 rows bounce through a
small Internal-DRAM scratch (DMA rejects stride-0 SBUF partition dims;
a flat DRAM row re-read 128x via [[0,128],[1,w]] is legal), costing PE
two fewer column-passes and ACT two fewer staging passes.  The broadcast
DMAs are issued BEFORE the 4.7MB dma_in prefetch so they don't queue
behind it (issuing them after measured +4us).
Previous checkpoint (v14): ~140.5us median (139.6-141.7 over 6 runs; occasional
~175us outliers under external device contention).  PE 92%-occupied at
135us busy — the engine-balance limit of this decomposition.
Session-3 additions on top of v7: tapered tile widths TW (small first
tile shortens the fill, small last tile + interleaved per-jslice drain
shortens the tail), and the final add+store deferred to finish_stage
after the next tile's narrow smalls so W4d(d) completes ~2us earlier and
the next iteration's PE broadcast never stalls on it.

v7 design:
 - Grids fp16 on HOST (HBM 23.7MB/core); output fp16, host upconverts.
 - DVE merge in fp16 2x_1p mode (0.52 ns/col, HW-verified): products as
   4 chunk-paired ops [128,2,1536] (wq broadcast over the chunk axis via
   an explicit 0-stride AP dim) + 3 paired adds, halving op count.
 - PE cost is passes x cols x 0.85ns (observed throttled clock) + 93ns
   LDWEIGHTS per matmul.  v4 ran 13 column passes on PE; v7 runs 10:
   grids 0-2's C-chunks are pre-folded on DVE (t_i = g_c0 + g_c1, fp16
   2x), so logits need 5 accumulating matmuls per jslice instead of 8.
   Folding all 4 would tip DVE past PE; 3 balances the two engines.
 - Broadcast staging via [128,768] PSUM tiles: per grid per 768-block,
   two matmuls (N=512+256, each within one PSUM bank) + ONE wide ACT
   copy PSUM->SBUF fp16 (halves v4's ACT op count and sem load).
 - Narrow tail per jslice: exp (ACT, scale=1/C exp-trick, bias=b_proj),
   S4 denominator (PE, ones lhsT), reciprocal_approx_fast (DVE, fp32),
   W4 = E*R4 -> fp16 into a d-tile-wide W4d (next iter's bcast rhs).
 - Per-iter emission (engine queue order is what matters):
     dma_in(d+1) | DVE folds(d) | PE bcast(d-1) + ACT staging | DVE
     products/adds(d-1) + dma_out(d-1) | narrow(d) | (DVE recip/W4 last)
   Folds go first on DVE so PE's logits(d) unblock early; products(d-1)
   keep DVE busy while the narrow(d) PE->ACT->PE chain round-trips.
 - The flush (last tile) runs broadcast+wide in two 768-col halves so the
   DVE wide work of half 1 overlaps the PE/ACT broadcast of half 2
   (-7us of serial drain).
 - Negative results (measured): all-gpsimd broadcast (fp32 partition_
   broadcast) 182-186us from SBUF-port DVE stalls; gpsimd partition_all_
   reduce for the softmax denominator 191us (3.5-3.9us/op + ~1us sem
   events on the Q7); fold-4 180us; 1024/512 bcast staging blocks 161us.
"""

import os
import sys

import numpy as np

try:
    import concourse.bass as bass
except ImportError:  # fresh grading dir: concourse lives in the container repo
    sys.path.insert(0, "/opt/trn_rl_repo")
    import concourse.bass as bass

from contextlib import ExitStack

import concourse.tile as tile
from concourse import bacc, mybir
from concourse.bass_utils import run_bass_kernel_spmd

B, C, H, W = 8, 256, 96, 96
HW = H * W  # 9216
NCORES = 8
CPB = C // 128  # 2 partition chunks per core
MAXW = 1536  # pool sizing; per-tile widths taper at both ends
# small first tile shortens the fill (dma+narrow before any wide work);
# small last tile shortens the drain (bcast+wide after the last narrow)
TW = [512, 1536, 1536, 1536, 1536, 1536, 1024]
assert sum(TW) == HW and all(w % 512 == 0 for w in TW)
ND = len(TW)
NFOLD = 3  # grids 0..NFOLD-1 chunk-folded on DVE for the logits pass
BLK = 768  # bcast staging block (2 matmuls + 1 ACT copy per grid)

F32 = mybir.dt.float32
F16 = mybir.dt.float16
AF = mybir.ActivationFunctionType

_CACHE = {}


def bcast2(ap, n):
    """AP broadcast over a new middle 'chunk' axis of size n (stride 0)."""
    return bass.AP(ap.tensor, ap.offset, [ap.ap[0], [0, n], ap.ap[-1]])


def build_program():
    nc = bacc.Bacc("TRN2", debug=False, num_devices=NCORES)

    gall_d = nc.dram_tensor("gall", [4, C, HW], F16, kind="ExternalInput").ap()
    # fp16 constants: cols 0-511 sel (one-hot bcast lhsT rows 0-3), cols
    # 512-527 ws (w[o,i] at col 512+4i+o, replicated down partitions),
    # cols 528-531 ones4x4
    ch_d = nc.dram_tensor("cblob16", [128, 532], F16, kind="ExternalInput").ap()
    # fp32 constants: col 0 rows 0-3 = b_proj
    cb_d = nc.dram_tensor("cblob", [128, 1], F32, kind="ExternalInput").ap()
    out = nc.dram_tensor("out", [C, HW], F16, kind="ExternalOutput").ap()
    # HBM bounce for the DMA-routed weight broadcast of grids 2,3: DMA
    # rejects stride-0 SBUF partition dims, but a flat DRAM source row can
    # be re-read 128x ([[0,128],[1,w]]), turning the broadcast into pure
    # DMA-engine work (no PE pass, no ACT staging).  Double-buffered by
    # d%2 (written end of narrow(d), read at iter d+1's bcast).
    wscr = nc.dram_tensor("wscr", [2, 2, MAXW], F16, kind="Internal").ap()

    with tile.TileContext(nc) as tc, ExitStack() as ctx:
        const = ctx.enter_context(tc.tile_pool(name="const", bufs=1))
        gin = ctx.enter_context(tc.tile_pool(name="gin", bufs=3))
        outp = ctx.enter_context(tc.tile_pool(name="outp", bufs=2))
        foldp = ctx.enter_context(tc.tile_pool(name="foldp", bufs=2))
        narrow = ctx.enter_context(tc.tile_pool(name="narrow", bufs=3))
        wqp = ctx.enter_context(tc.tile_pool(name="wqp", bufs=2))
        prod = ctx.enter_context(tc.tile_pool(name="prod", bufs=3))
        qpool = ctx.enter_context(tc.tile_pool(name="qpool", bufs=3))
        ps_nar = ctx.enter_context(tc.tile_pool(name="psnar", bufs=2, space="PSUM"))
        ps_wb = ctx.enter_context(tc.tile_pool(name="pswb", bufs=2, space="PSUM"))

        ch = const.tile([128, 532], F16)
        nc.sync.dma_start(out=ch[:], in_=ch_d)
        cb = const.tile([128, 1], F32)
        nc.sync.dma_start(out=cb[:], in_=cb_d)
        bv = cb[0:4, 0:1]

        def ws_i(i):  # [128, 4] logits lhsT for grid i
            return ch[:, 512 + 4 * i : 512 + 4 * i + 4]

        def sel_i(i):  # [4, 128] bcast lhsT for grid i
            return ch[0:4, 128 * i : 128 * (i + 1)]

        ones4 = ch[0:4, 528:532]

        # Warmup matmul: absorbs the const-blob DMA wait on the PE clock.
        warm = ps_nar.tile([128, 512], F32, tag="smx")
        nc.tensor.matmul(warm[0:4, 0:16], lhsT=ch[0:4, 0:4], rhs=ch[0:4, 0:16],
                         start=True, stop=True)

        def fold_stage(gat, w):
            """DVE chunk-fold for grids 0..NFOLD-1 (fp16 2x): the logits
            contraction over (grid, chunk) shrinks from 8 to 5 matmuls."""
            ts = []
            for i in range(NFOLD):
                t = foldp.tile([128, MAXW], F16, tag=f"t{i}")
                nc.vector.tensor_add(t[:, 0:w], gat[:, i, 0, 0:w],
                                     gat[:, i, 1, 0:w])
                ts.append(t)
            return ts

        def narrow_stage(d, gat, ts):
            """Softmax chain: logits (jslice pairs) -> exp -> S4 -> recip
            -> W4 (fp16, d-tile-wide for next iter's bcast rhs).  Matmul
            outputs at PSUM base partition 0 (ISA constraint)."""
            jslc = [(x0, 512) for x0 in range(0, TW[d], 512)]
            W4d = narrow.tile([4, MAXW], F16, tag="W4", bufs=2)
            for pair in [jslc[k : k + 2] for k in range(0, len(jslc), 2)]:
                Ls, Es = [], []
                for x0, n in pair:
                    L = ps_nar.tile([128, 512], F32, tag="smx")
                    Ls.append(L[0:4, 0:n])
                    rhss = [ts[i][:, x0 : x0 + n] for i in range(NFOLD)]
                    rhss += [
                        gat[:, i, c, x0 : x0 + n]
                        for i in range(NFOLD, 4)
                        for c in range(CPB)
                    ]
                    lhss = [ws_i(i) for i in range(NFOLD)] + [
                        ws_i(i) for i in range(NFOLD, 4) for _ in range(CPB)
                    ]
                    for k, (lh, rh) in enumerate(zip(lhss, rhss)):
                        nc.tensor.matmul(Ls[-1], lhsT=lh, rhs=rh,
                                         start=(k == 0),
                                         stop=(k == len(rhss) - 1))
                for pi, (x0, n) in enumerate(pair):
                    E = narrow.tile([4, 512], F16, tag="E")
                    nc.scalar.activation(E[0:4, 0:n], Ls[pi], AF.Exp,
                                         bias=bv, scale=1.0 / C)
                    Es.append(E[0:4, 0:n])
                S4s = []
                for pi, (x0, n) in enumerate(pair):
                    S4 = ps_nar.tile([4, 512], F32, tag="S4")
                    nc.tensor.matmul(S4[0:4, 0:n], lhsT=ones4, rhs=Es[pi],
                                     start=True, stop=True)
                    S4s.append(S4[0:4, 0:n])
                for pi, (x0, n) in enumerate(pair):
                    R4 = narrow.tile([4, 512], F32, tag="R4", bufs=2)
                    nc.vector.reciprocal_approx_fast(R4[0:4, 0:n], S4s[pi])
                    nc.vector.tensor_mul(W4d[0:4, x0 : x0 + n], Es[pi],
                                         R4[0:4, 0:n])
            nc.sync.dma_start(out=wscr[d % 2, :, 0 : TW[d]],
                              in_=W4d[2:4, 0 : TW[d]])
            return W4d

        def bcast_stage(prev):
            """PE broadcast of W4 rows to 128 partitions, staged to fp16
            SBUF by wide [128,768] ACT copies (each matmul writes within a
            single PSUM bank; the copy spans banks, reads are unrestricted)."""
            if prev is None:
                return None
            d, gat, ot, W4d = prev
            w = TW[d]
            wq = {}
            for i in range(4):
                wqt = wqp.tile([128, MAXW], F16, tag=f"wq{i}")
                wq[i] = wqt
            for i in (2, 3):
                row = wscr[d % 2, i - 2 : i - 1, 0:w]
                src = bass.AP(row.tensor, row.offset, [[0, 128], [1, w]])
                nc.sync.dma_start(out=wq[i][:, 0:w], in_=src)
            for b0 in range(0, w, BLK):
                bw = min(BLK, w - b0)
                for i in (0, 1):
                    Wb = ps_wb.tile([128, BLK], F32, tag="wb")
                    for s0 in range(0, bw, 512):
                        n = min(512, bw - s0)
                        nc.tensor.matmul(
                            Wb[:, s0 : s0 + n],
                            lhsT=sel_i(i),
                            rhs=W4d[0:4, b0 + s0 : b0 + s0 + n],
                            start=True, stop=True,
                        )
                    nc.scalar.copy(wq[i][:, b0 : b0 + bw], Wb[:, 0:bw])
            return (d, gat, ot, wq)

        def wide_stage(staged):
            """DVE products + first add-tree level (chunk-paired ops).
            The final add + store happen in finish_stage AFTER the next
            tile's narrow smalls, so W4d(d) completes ~2us earlier and the
            next iteration's PE broadcast doesn't stall on it."""
            if staged is None:
                return None
            d, gat, ot, wq = staged
            w = TW[d]
            p = {}
            for i in range(4):
                pt = prod.tile([128, CPB, MAXW], F16, tag="p")
                nc.vector.tensor_mul(pt[:, :, 0:w], gat[:, i, :, 0:w],
                                     bcast2(wq[i][:, 0:w], CPB))
                p[i] = pt
                if i == 1:
                    q01 = qpool.tile([128, CPB, MAXW], F16, tag="q")
                    nc.vector.tensor_add(q01[:, :, 0:w], p[0][:, :, 0:w],
                                         p[1][:, :, 0:w])
            q23 = qpool.tile([128, CPB, MAXW], F16, tag="q")
            nc.vector.tensor_add(q23[:, :, 0:w], p[2][:, :, 0:w],
                                 p[3][:, :, 0:w])
            return (d, ot, q01, q23)

        def finish_stage(pend):
            if pend is None:
                return
            d, ot, q01, q23 = pend
            w = TW[d]
            nc.vector.tensor_add(ot[:, :, 0:w], q01[:, :, 0:w],
                                 q23[:, :, 0:w])
            n0 = sum(TW[:d])
            nc.sync.dma_start(
                out=out[:, n0 : n0 + w].rearrange("(c p) n -> p c n", c=CPB),
                in_=ot[:, :, 0:w],
            )

        def dma_in(d):
            n0 = sum(TW[:d])
            w = TW[d]
            gat = gin.tile([128, 4, CPB, MAXW], F16, tag="gall")
            nc.sync.dma_start(
                out=gat[:, :, :, 0:w],
                in_=gall_d[:, :, n0 : n0 + w].rearrange(
                    "i (c p) n -> p i c n", c=CPB
                ),
            )
            return gat

        gats = {0: dma_in(0)}
        prev = None  # (d, gat, ot, W4d) awaiting bcast+wide
        for d in range(ND - 1):
            gat = gats.pop(d)
            ts = fold_stage(gat, TW[d])
            # bcast first: its (tiny) broadcast DMAs must hit the sync queue
            # ahead of the 4.7MB dma_in so wq2/wq3 aren't delayed behind it
            staged = bcast_stage(prev)
            if d + 1 < ND:
                gats[d + 1] = dma_in(d + 1)
            pend = wide_stage(staged)
            ot = outp.tile([128, CPB, MAXW], F16, tag="ot")
            W4d = narrow_stage(d, gat, ts)
            finish_stage(pend)
            prev = (d, gat, ot, W4d)
        # Last tile: single-jslice narrow chains with the drain interleaved
        # per slice.  Each slice's recip/W4 feeds its drain broadcast
        # immediately, so the PE/ACT drain work overlaps the other slice's
        # DVE work instead of serializing after the whole narrow chain.
        d = ND - 1
        gat = gats.pop(d)
        ts = fold_stage(gat, TW[d])
        staged = bcast_stage(prev)
        finish_stage(wide_stage(staged))
        ot = outp.tile([128, CPB, MAXW], F16, tag="ot")
        w = TW[d]
        jslc = [(x0, 512) for x0 in range(0, w, 512)]
        W4d = narrow.tile([4, MAXW], F16, tag="W4", bufs=2)
        chain = []
        for x0, n in jslc:
            L = ps_nar.tile([128, 512], F32, tag="smx")
            Lv = L[0:4, 0:n]
            rhss = [ts[i][:, x0 : x0 + n] for i in range(NFOLD)] + [
                gat[:, i, c, x0 : x0 + n]
                for i in range(NFOLD, 4)
                for c in range(CPB)
            ]
            lhss = [ws_i(i) for i in range(NFOLD)] + [
                ws_i(i) for i in range(NFOLD, 4) for _ in range(CPB)
            ]
            for k, (lh, rh) in enumerate(zip(lhss, rhss)):
                nc.tensor.matmul(Lv, lhsT=lh, rhs=rh, start=(k == 0),
                                 stop=(k == len(rhss) - 1))
            E = narrow.tile([4, 512], F16, tag="E")
            nc.scalar.activation(E[0:4, 0:n], Lv, AF.Exp, bias=bv,
                                 scale=1.0 / C)
            S4 = ps_nar.tile([4, 512], F32, tag="S4")
            nc.tensor.matmul(S4[0:4, 0:n], lhsT=ones4, rhs=E[0:4, 0:n],
                             start=True, stop=True)
            chain.append((x0, n, E, S4))
        for x0, n, E, S4 in chain:
            R4 = narrow.tile([4, 512], F32, tag="R4", bufs=2)
            nc.vector.reciprocal_approx_fast(R4[0:4, 0:n], S4[0:4, 0:n])
            nc.vector.tensor_mul(W4d[0:4, x0 : x0 + n], E[0:4, 0:n],
                                 R4[0:4, 0:n])
        dr = []
        for x0, n, E, S4 in chain:
            wqh = {}
            for i in range(4):
                wqt = wqp.tile([128, MAXW], F16, tag=f"wq{i}")
                wqh[i] = wqt
                Wb = ps_wb.tile([128, BLK], F32, tag="wb")
                nc.tensor.matmul(Wb[:, 0:n], lhsT=sel_i(i),
                                 rhs=W4d[0:4, x0 : x0 + n],
                                 start=True, stop=True)
                nc.scalar.copy(wqt[:, x0 : x0 + n], Wb[:, 0:n])
            dr.append((x0, n, wqh))
        for x0, n, wqh in dr:
            p = {}
            for i in range(4):
                pt = prod.tile([128, CPB, MAXW], F16, tag="p")
                nc.vector.tensor_mul(
                    pt[:, :, 0:n], gat[:, i, :, x0 : x0 + n],
                    bcast2(wqh[i][:, x0 : x0 + n], CPB),
                )
                p[i] = pt
                if i == 1:
                    q01 = qpool.tile([128, CPB, MAXW], F16, tag="q")
                    nc.vector.tensor_add(q01[:, :, 0:n], p[0][:, :, 0:n],
                                         p[1][:, :, 0:n])
            q23 = qpool.tile([128, CPB, MAXW], F16, tag="q")
            nc.vector.tensor_add(q23[:, :, 0:n], p[2][:, :, 0:n],
                                 p[3][:, :, 0:n])
            nc.vector.tensor_add(ot[:, :, x0 : x0 + n], q01[:, :, 0:n],
                                 q23[:, :, 0:n])
            n0 = sum(TW[:d]) + x0
            nc.sync.dma_start(
                out=out[:, n0 : n0 + n].rearrange("(c p) n -> p c n", c=CPB),
                in_=ot[:, :, x0 : x0 + n],
            )

    nc.compile()
    return nc


def _get_program():
    if "nc" not in _CACHE:
        _CACHE["nc"] = build_program()
    return _CACHE["nc"]


def make_cblobs(w_proj, b_proj):
    w = np.asarray(w_proj, dtype=np.float32)
    b = np.asarray(b_proj, dtype=np.float32)
    ch = np.zeros((128, 532), dtype=np.float16)
    sel = np.repeat(np.eye(4, dtype=np.float16), 128, axis=1)
    ch[0:4, 0:512] = sel
    for i in range(4):
        for o in range(4):
            ch[:, 512 + 4 * i + o] = np.float16(w[o, i])
    ch[0:4, 528:532] = 1.0
    cb = np.zeros((128, 1), dtype=np.float32)
    cb[0:4, 0] = b
    return ch, cb


LAST_RESULT = None


def kernel(g0, g1, g2, g3, w_proj, b_proj):
    global LAST_RESULT
    nc = _get_program()

    ch, cb = make_cblobs(w_proj, b_proj)

    gall = np.stack(
        [np.asarray(x).reshape(B, C, HW).astype(np.float16) for x in (g0, g1, g2, g3)],
        axis=1,
    )  # (B, 4, C, HW) fp16
    in_maps = []
    for bi in range(NCORES):
        m = {"gall": np.ascontiguousarray(gall[bi]), "cblob16": ch, "cblob": cb}
        in_maps.append(m)

    res = run_bass_kernel_spmd(
        nc,
        in_maps,
        list(range(NCORES)),
        trace=bool(int(os.environ.get("CM_TRACE", "0"))),
        tmpdir=os.environ.get("CM_TRACE_DIR") or None,
    )
    LAST_RESULT = res
    out_full = np.stack(
        [
            res.results[bi]["out"].astype(np.float32).reshape(C, H, W)
            for bi in range(NCORES)
        ],
        axis=0,
    )
    return out_full


# revision 55
# speedup vs baseline: 1.1606x; 1.0900x over previous
"""CrossMerge kernel for trn2 — v7 (fp16, PE broadcast, DVE chunk-folding).

Math (per batch element):
    means_i = mean over C of g_i              (4, H, W)
    logits  = w_proj @ means + b_proj         (4, H, W)
    w       = softmax(logits, axis=0)         (4, H, W)
    out     = sum_i g_i * w_i                 (C, H, W)

Sharding: data-parallel over batch B=8 across 8 cores; weights replicated;
no cross-device communication.

Measured history: v3 fp32 247us; v4 fp16 157.5us (PE-bound 148us busy);
v5b/v6 gpsimd-broadcast variants 182-186us — the gpsimd fp32 broadcast
writes 19-38MB through the SBUF ports and stalls concurrent DVE ops to
3.1us/op (vs their 952ns median), so broadcasts stay on PE+PSUM (PSUM has
its own ports; the ACT staging writes only the final 9.4MB of fp16).

Final (v16r) measured: ~139-142us (best 138.3; adjacent-run A/B beat v14
by ~1.6us).  Session-5 addition: grids 2,3's weight broadcast is routed
through the DMA engines instead of PE+ACT — W4 rows bounce through a
small Internal-DRAM scratch (DMA rejects stride-0 SBUF partition dims;
a flat DRAM row re-read 128x via [[0,128],[1,w]] is legal), costing PE
two fewer column-passes and ACT two fewer staging passes.  The broadcast
DMAs are issued BEFORE the 4.7MB dma_in prefetch so they don't queue
behind it (issuing them after measured +4us).
Previous checkpoint (v14): ~140.5us median (139.6-141.7 over 6 runs; occasional
~175us outliers under external device contention).  PE 92%-occupied at
135us busy — the engine-balance limit of this decomposition.
Session-3 additions on top of v7: tapered tile widths TW (small first
tile shortens the fill, small last tile + interleaved per-jslice drain
shortens the tail), and the final add+store deferred to finish_stage
after the next tile's narrow smalls so W4d(d) completes ~2us earlier and
the next iteration's PE broadcast never stalls on it.

v7 design:
 - Grids fp16 on HOST (HBM 23.7MB/core); output fp16, host upconverts.
 - DVE merge in fp16 2x_1p mode (0.52 ns/col, HW-verified): products as
   4 chunk-paired ops [128,2,1536] (wq broadcast over the chunk axis via
   an explicit 0-stride AP dim) + 3 paired adds, halving op count.
 - PE cost is passes x cols x 0.85ns (observed throttled clock) + 93ns
   LDWEIGHTS per matmul.  v4 ran 13 column passes on PE; v7 runs 10:
   grids 0-2's C-chunks are pre-folded on DVE (t_i = g_c0 + g_c1, fp16
   2x), so logits need 5 accumulating matmuls per jslice instead of 8.
   Folding all 4 would tip DVE past PE; 3 balances the two engines.
 - Broadcast staging via [128,768] PSUM tiles: per grid per 768-block,
   two matmuls (N=512+256, each within one PSUM bank) + ONE wide ACT
   copy PSUM->SBUF fp16 (halves v4's ACT op count and sem load).
 - Narrow tail per jslice: exp (ACT, scale=1/C exp-trick, bias=b_proj),
   S4 denominator (PE, ones lhsT), reciprocal_approx_fast (DVE, fp32),
   W4 = E*R4 -> fp16 into a d-tile-wide W4d (next iter's bcast rhs).
 - Per-iter emission (engine queue order is what matters):
     dma_in(d+1) | DVE folds(d) | PE bcast(d-1) + ACT staging | DVE
     products/adds(d-1) + dma_out(d-1) | narrow(d) | (DVE recip/W4 last)
   Folds go first on DVE so PE's logits(d) unblock early; products(d-1)
   keep DVE busy while the narrow(d) PE->ACT->PE chain round-trips.
 - The flush (last tile) runs broadcast+wide in two 768-col halves so the
   DVE wide work of half 1 overlaps the PE/ACT broadcast of half 2
   (-7us of serial drain).
 - Negative results (measured): all-gpsimd broadcast (fp32 partition_
   broadcast) 182-186us from SBUF-port DVE stalls; gpsimd partition_all_
   reduce for the softmax denominator 191us (3.5-3.9us/op + ~1us sem
   events on the Q7); fold-4 180us; 1024/512 bcast staging blocks 161us.
"""

import os
import sys

import numpy as np

try:
    import concourse.bass as bass
except ImportError:  # fresh grading dir: concourse lives in the container repo
    sys.path.insert(0, "/opt/trn_rl_repo")
    import concourse.bass as bass

from contextlib import ExitStack

import concourse.tile as tile
from concourse import bacc, mybir
from concourse.bass_utils import run_bass_kernel_spmd

B, C, H, W = 8, 256, 96, 96
HW = H * W  # 9216
NCORES = 8
CPB = C // 128  # 2 partition chunks per core
MAXW = 1536  # pool sizing; per-tile widths taper at both ends
# small first tile shortens the fill (dma+narrow before any wide work);
# small last tile shortens the drain (bcast+wide after the last narrow)
TW = [512, 1536, 1536, 1536, 1536, 1536, 1024]
assert sum(TW) == HW and all(w % 512 == 0 for w in TW)
ND = len(TW)
NFOLD = 3  # grids 0..NFOLD-1 chunk-folded on DVE for the logits pass
BLK = 768  # bcast staging block (2 matmuls + 1 ACT copy per grid)

F32 = mybir.dt.float32
F16 = mybir.dt.float16
AF = mybir.ActivationFunctionType

_CACHE = {}


def bcast2(ap, n):
    """AP broadcast over a new middle 'chunk' axis of size n (stride 0)."""
    return bass.AP(ap.tensor, ap.offset, [ap.ap[0], [0, n], ap.ap[-1]])


def build_program():
    nc = bacc.Bacc("TRN2", debug=False, num_devices=NCORES)

    gall_d = nc.dram_tensor("gall", [4, C, HW], F16, kind="ExternalInput").ap()
    # fp16 constants: cols 0-511 sel (one-hot bcast lhsT rows 0-3), cols
    # 512-527 ws (w[o,i] at col 512+4i+o, replicated down partitions),
    # cols 528-531 ones4x4
    ch_d = nc.dram_tensor("cblob16", [128, 532], F16, kind="ExternalInput").ap()
    # fp32 constants: col 0 rows 0-3 = b_proj
    cb_d = nc.dram_tensor("cblob", [128, 1], F32, kind="ExternalInput").ap()
    out = nc.dram_tensor("out", [C, HW], F16, kind="ExternalOutput").ap()
    # HBM bounce for the DMA-routed weight broadcast of grids 2,3: DMA
    # rejects stride-0 SBUF partition dims, but a flat DRAM source row can
    # be re-read 128x ([[0,128],[1,w]]), turning the broadcast into pure
    # DMA-engine work (no PE pass, no ACT staging).  Double-buffered by
    # d%2 (written end of narrow(d), read at iter d+1's bcast).
    wscr = nc.dram_tensor("wscr", [2, 2, MAXW], F16, kind="Internal").ap()

    with tile.TileContext(nc) as tc, ExitStack() as ctx:
        const = ctx.enter_context(tc.tile_pool(name="const", bufs=1))
        gin = ctx.enter_context(tc.tile_pool(name="gin", bufs=3))
        outp = ctx.enter_context(tc.tile_pool(name="outp", bufs=2))
        foldp = ctx.enter_context(tc.tile_pool(name="foldp", bufs=2))
        narrow = ctx.enter_context(tc.tile_pool(name="narrow", bufs=3))
        wqp = ctx.enter_context(tc.tile_pool(name="wqp", bufs=2))
        prod = ctx.enter_context(tc.tile_pool(name="prod", bufs=3))
        qpool = ctx.enter_context(tc.tile_pool(name="qpool", bufs=3))
        ps_nar = ctx.enter_context(tc.tile_pool(name="psnar", bufs=2, space="PSUM"))
        ps_wb = ctx.enter_context(tc.tile_pool(name="pswb", bufs=2, space="PSUM"))

        ch = const.tile([128, 532], F16)
        nc.sync.dma_start(out=ch[:], in_=ch_d)
        cb = const.tile([128, 1], F32)
        nc.sync.dma_start(out=cb[:], in_=cb_d)
        bv = cb[0:4, 0:1]

        def ws_i(i):  # [128, 4] logits lhsT for grid i
            return ch[:, 512 + 4 * i : 512 + 4 * i + 4]

        def sel_i(i):  # [4, 128] bcast lhsT for grid i
            return ch[0:4, 128 * i : 128 * (i + 1)]

        ones4 = ch[0:4, 528:532]

        # Warmup matmul: absorbs the const-blob DMA wait on the PE clock.
        warm = ps_nar.tile([128, 512], F32, tag="smx")
        nc.tensor.matmul(warm[0:4, 0:16], lhsT=ch[0:4, 0:4], rhs=ch[0:4, 0:16],
                         start=True, stop=True)

        def fold_stage(gat, w):
            """DVE chunk-fold for grids 0..NFOLD-1 (fp16 2x): the logits
            contraction over (grid, chunk) shrinks from 8 to 5 matmuls."""
            ts = []
            for i in range(NFOLD):
                t = foldp.tile([128, MAXW], F16, tag=f"t{i}")
                nc.vector.tensor_add(t[:, 0:w], gat[:, i, 0, 0:w],
                                     gat[:, i, 1, 0:w])
                ts.append(t)
            return ts

        def narrow_stage(d, gat, ts):
            """Softmax chain: logits (jslice pairs) -> exp -> S4 -> recip
            -> W4 (fp16, d-tile-wide for next iter's bcast rhs).  Matmul
            outputs at PSUM base partition 0 (ISA constraint)."""
            jslc = [(x0, 512) for x0 in range(0, TW[d], 512)]
            W4d = narrow.tile([4, MAXW], F16, tag="W4", bufs=2)
            for pair in [jslc[k : k + 2] for k in range(0, len(jslc), 2)]:
                Ls, Es = [], []
                for x0, n in pair:
                    L = ps_nar.tile([128, 512], F32, tag="smx")
                    Ls.append(L[0:4, 0:n])
                    rhss = [ts[i][:, x0 : x0 + n] for i in range(NFOLD)]
                    rhss += [
                        gat[:, i, c, x0 : x0 + n]
                        for i in range(NFOLD, 4)
                        for c in range(CPB)
                    ]
                    lhss = [ws_i(i) for i in range(NFOLD)] + [
                        ws_i(i) for i in range(NFOLD, 4) for _ in range(CPB)
                    ]
                    for k, (lh, rh) in enumerate(zip(lhss, rhss)):
                        nc.tensor.matmul(Ls[-1], lhsT=lh, rhs=rh,
                                         start=(k == 0),
                                         stop=(k == len(rhss) - 1))
                for pi, (x0, n) in enumerate(pair):
                    E = narrow.tile([4, 512], F16, tag="E")
                    nc.scalar.activation(E[0:4, 0:n], Ls[pi], AF.Exp,
                                         bias=bv, scale=1.0 / C)
                    Es.append(E[0:4, 0:n])
                S4s = []
                for pi, (x0, n) in enumerate(pair):
                    S4 = ps_nar.tile([4, 512], F32, tag="S4")
                    nc.tensor.matmul(S4[0:4, 0:n], lhsT=ones4, rhs=Es[pi],
                                     start=True, stop=True)
                    S4s.append(S4[0:4, 0:n])
                for pi, (x0, n) in enumerate(pair):
                    R4 = narrow.tile([4, 512], F32, tag="R4", bufs=2)
                    nc.vector.reciprocal_approx_fast(R4[0:4, 0:n], S4s[pi])
                    nc.vector.tensor_mul(W4d[0:4, x0 : x0 + n], Es[pi],
                                         R4[0:4, 0:n])
            # gpsimd-issued: same queue as the broadcast reads (FIFO keeps
            # the DRAM RAW hazard ordered) and off the busy sync queue
            nc.gpsimd.dma_start(out=wscr[d % 2, :, 0 : TW[d]],
                                in_=W4d[2:4, 0 : TW[d]])
            return W4d

        def bcast_stage(prev):
            """PE broadcast of W4 rows to 128 partitions, staged to fp16
            SBUF by wide [128,768] ACT copies (each matmul writes within a
            single PSUM bank; the copy spans banks, reads are unrestricted)."""
            if prev is None:
                return None
            d, gat, ot, W4d = prev
            w = TW[d]
            wq = {}
            for i in range(4):
                wqt = wqp.tile([128, MAXW], F16, tag=f"wq{i}")
                wq[i] = wqt
            for i in (2, 3):
                row = wscr[d % 2, i - 2 : i - 1, 0:w]
                src = bass.AP(row.tensor, row.offset, [[0, 128], [1, w]])
                nc.gpsimd.dma_start(out=wq[i][:, 0:w], in_=src)
            for b0 in range(0, w, BLK):
                bw = min(BLK, w - b0)
                for i in (0, 1):
                    Wb = ps_wb.tile([128, BLK], F32, tag="wb")
                    for s0 in range(0, bw, 512):
                        n = min(512, bw - s0)
                        nc.tensor.matmul(
                            Wb[:, s0 : s0 + n],
                            lhsT=sel_i(i),
                            rhs=W4d[0:4, b0 + s0 : b0 + s0 + n],
                            start=True, stop=True,
                        )
                    nc.scalar.copy(wq[i][:, b0 : b0 + bw], Wb[:, 0:bw])
            return (d, gat, ot, wq)

        def wide_stage(staged):
            """DVE products + first add-tree level (chunk-paired ops).
            The final add + store happen in finish_stage AFTER the next
            tile's narrow smalls, so W4d(d) completes ~2us earlier and the
            next iteration's PE broadcast doesn't stall on it."""
            if staged is None:
                return None
            d, gat, ot, wq = staged
            w = TW[d]
            p = {}
            for i in range(4):
                pt = prod.tile([128, CPB, MAXW], F16, tag="p")
                nc.vector.tensor_mul(pt[:, :, 0:w], gat[:, i, :, 0:w],
                                     bcast2(wq[i][:, 0:w], CPB))
                p[i] = pt
                if i == 1:
                    q01 = qpool.tile([128, CPB, MAXW], F16, tag="q")
                    nc.vector.tensor_add(q01[:, :, 0:w], p[0][:, :, 0:w],
                                         p[1][:, :, 0:w])
            q23 = qpool.tile([128, CPB, MAXW], F16, tag="q")
            nc.vector.tensor_add(q23[:, :, 0:w], p[2][:, :, 0:w],
                                 p[3][:, :, 0:w])
            return (d, ot, q01, q23)

        def finish_stage(pend):
            if pend is None:
                return
            d, ot, q01, q23 = pend
            w = TW[d]
            nc.vector.tensor_add(ot[:, :, 0:w], q01[:, :, 0:w],
                                 q23[:, :, 0:w])
            n0 = sum(TW[:d])
            nc.sync.dma_start(
                out=out[:, n0 : n0 + w].rearrange("(c p) n -> p c n", c=CPB),
                in_=ot[:, :, 0:w],
            )

        def dma_in(d):
            n0 = sum(TW[:d])
            w = TW[d]
            gat = gin.tile([128, 4, CPB, MAXW], F16, tag="gall")
            nc.sync.dma_start(
                out=gat[:, :, :, 0:w],
                in_=gall_d[:, :, n0 : n0 + w].rearrange(
                    "i (c p) n -> p i c n", c=CPB
                ),
            )
            return gat

        gats = {0: dma_in(0)}
        prev = None  # (d, gat, ot, W4d) awaiting bcast+wide
        for d in range(ND - 1):
            gat = gats.pop(d)
            ts = fold_stage(gat, TW[d])
            # bcast first: its (tiny) broadcast DMAs must hit the sync queue
            # ahead of the 4.7MB dma_in so wq2/wq3 aren't delayed behind it
            staged = bcast_stage(prev)
            if d + 1 < ND:
                gats[d + 1] = dma_in(d + 1)
            pend = wide_stage(staged)
            ot = outp.tile([128, CPB, MAXW], F16, tag="ot")
            W4d = narrow_stage(d, gat, ts)
            finish_stage(pend)
            prev = (d, gat, ot, W4d)
        # Last tile: single-jslice narrow chains with the drain interleaved
        # per slice.  Each slice's recip/W4 feeds its drain broadcast
        # immediately, so the PE/ACT drain work overlaps the other slice's
        # DVE work instead of serializing after the whole narrow chain.
        d = ND - 1
        gat = gats.pop(d)
        ts = fold_stage(gat, TW[d])
        staged = bcast_stage(prev)
        finish_stage(wide_stage(staged))
        ot = outp.tile([128, CPB, MAXW], F16, tag="ot")
        w = TW[d]
        jslc = [(x0, 512) for x0 in range(0, w, 512)]
        W4d = narrow.tile([4, MAXW], F16, tag="W4", bufs=2)
        chain = []
        for x0, n in jslc:
            L = ps_nar.tile([128, 512], F32, tag="smx")
            Lv = L[0:4, 0:n]
            rhss = [ts[i][:, x0 : x0 + n] for i in range(NFOLD)] + [
                gat[:, i, c, x0 : x0 + n]
                for i in range(NFOLD, 4)
                for c in range(CPB)
            ]
            lhss = [ws_i(i) for i in range(NFOLD)] + [
                ws_i(i) for i in range(NFOLD, 4) for _ in range(CPB)
            ]
            for k, (lh, rh) in enumerate(zip(lhss, rhss)):
                nc.tensor.matmul(Lv, lhsT=lh, rhs=rh, start=(k == 0),
                                 stop=(k == len(rhss) - 1))
            E = narrow.tile([4, 512], F16, tag="E")
            nc.scalar.activation(E[0:4, 0:n], Lv, AF.Exp, bias=bv,
                                 scale=1.0 / C)
            S4 = ps_nar.tile([4, 512], F32, tag="S4")
            nc.tensor.matmul(S4[0:4, 0:n], lhsT=ones4, rhs=E[0:4, 0:n],
                             start=True, stop=True)
            chain.append((x0, n, E, S4))
        for x0, n, E, S4 in chain:
            R4 = narrow.tile([4, 512], F32, tag="R4", bufs=2)
            nc.vector.reciprocal_approx_fast(R4[0:4, 0:n], S4[0:4, 0:n])
            nc.vector.tensor_mul(W4d[0:4, x0 : x0 + n], E[0:4, 0:n],
                                 R4[0:4, 0:n])
        dr = []
        for x0, n, E, S4 in chain:
            wqh = {}
            for i in range(4):
                wqt = wqp.tile([128, MAXW], F16, tag=f"wq{i}")
                wqh[i] = wqt
                Wb = ps_wb.tile([128, BLK], F32, tag="wb")
                nc.tensor.matmul(Wb[:, 0:n], lhsT=sel_i(i),
                                 rhs=W4d[0:4, x0 : x0 + n],
                                 start=True, stop=True)
                nc.scalar.copy(wqt[:, x0 : x0 + n], Wb[:, 0:n])
            dr.append((x0, n, wqh))
        for x0, n, wqh in dr:
            p = {}
            for i in range(4):
                pt = prod.tile([128, CPB, MAXW], F16, tag="p")
                nc.vector.tensor_mul(
                    pt[:, :, 0:n], gat[:, i, :, x0 : x0 + n],
                    bcast2(wqh[i][:, x0 : x0 + n], CPB),
                )
                p[i] = pt
                if i == 1:
                    q01 = qpool.tile([128, CPB, MAXW], F16, tag="q")
                    nc.vector.tensor_add(q01[:, :, 0:n], p[0][:, :, 0:n],
                                         p[1][:, :, 0:n])
            q23 = qpool.tile([128, CPB, MAXW], F16, tag="q")
            nc.vector.tensor_add(q23[:, :, 0:n], p[2][:, :, 0:n],
                                 p[3][:, :, 0:n])
            nc.vector.tensor_add(ot[:, :, x0 : x0 + n], q01[:, :, 0:n],
                                 q23[:, :, 0:n])
            n0 = sum(TW[:d]) + x0
            nc.sync.dma_start(
                out=out[:, n0 : n0 + n].rearrange("(c p) n -> p c n", c=CPB),
                in_=ot[:, :, x0 : x0 + n],
            )

    nc.compile()
    return nc


def _get_program():
    if "nc" not in _CACHE:
        _CACHE["nc"] = build_program()
    return _CACHE["nc"]


def make_cblobs(w_proj, b_proj):
    w = np.asarray(w_proj, dtype=np.float32)
    b = np.asarray(b_proj, dtype=np.float32)
    ch = np.zeros((128, 532), dtype=np.float16)
    sel = np.repeat(np.eye(4, dtype=np.float16), 128, axis=1)
    ch[0:4, 0:512] = sel
    for i in range(4):
        for o in range(4):
            ch[:, 512 + 4 * i + o] = np.float16(w[o, i])
    ch[0:4, 528:532] = 1.0
    cb = np.zeros((128, 1), dtype=np.float32)
    cb[0:4, 0] = b
    return ch, cb


LAST_RESULT = None


def kernel(g0, g1, g2, g3, w_proj, b_proj):
    global LAST_RESULT
    nc = _get_program()

    ch, cb = make_cblobs(w_proj, b_proj)

    gall = np.stack(
        [np.asarray(x).reshape(B, C, HW).astype(np.float16) for x in (g0, g1, g2, g3)],
        axis=1,
    )  # (B, 4, C, HW) fp16
    in_maps = []
    for bi in range(NCORES):
        m = {"gall": np.ascontiguousarray(gall[bi]), "cblob16": ch, "cblob": cb}
        in_maps.append(m)

    res = run_bass_kernel_spmd(
        nc,
        in_maps,
        list(range(NCORES)),
        trace=bool(int(os.environ.get("CM_TRACE", "0"))),
        tmpdir=os.environ.get("CM_TRACE_DIR") or None,
    )
    LAST_RESULT = res
    out_full = np.stack(
        [
            res.results[bi]["out"].astype(np.float32).reshape(C, H, W)
            for bi in range(NCORES)
        ],
        axis=0,
    )
    return out_full


# revision 56
# speedup vs baseline: 1.2863x; 1.1084x over previous
"""CrossMerge kernel for trn2 — v7 (fp16, PE broadcast, DVE chunk-folding).

Math (per batch element):
    means_i = mean over C of g_i              (4, H, W)
    logits  = w_proj @ means + b_proj         (4, H, W)
    w       = softmax(logits, axis=0)         (4, H, W)
    out     = sum_i g_i * w_i                 (C, H, W)

Sharding: data-parallel over batch B=8 across 8 cores; weights replicated;
no cross-device communication.

Measured history: v3 fp32 247us; v4 fp16 157.5us (PE-bound 148us busy);
v5b/v6 gpsimd-broadcast variants 182-186us — the gpsimd fp32 broadcast
writes 19-38MB through the SBUF ports and stalls concurrent DVE ops to
3.1us/op (vs their 952ns median), so broadcasts stay on PE+PSUM (PSUM has
its own ports; the ACT staging writes only the final 9.4MB of fp16).

Final (v16r) measured: ~139-142us (best 138.3; adjacent-run A/B beat v14
by ~1.6us).  Session-5 addition: grids 2,3's weight broadcast is routed
through the DMA engines instead of PE+ACT — W4 rows bounce through a
small Internal-DRAM scratch (DMA rejects stride-0 SBUF partition dims;
a flat DRAM row re-read 128x via [[0,128],[1,w]] is legal), costing PE
two fewer column-passes and ACT two fewer staging passes.  The broadcast
DMAs are issued BEFORE the 4.7MB dma_in prefetch so they don't queue
behind it (issuing them after measured +4us).
Previous checkpoint (v14): ~140.5us median (139.6-141.7 over 6 runs; occasional
~175us outliers under external device contention).  PE 92%-occupied at
135us busy — the engine-balance limit of this decomposition.
Session-3 additions on top of v7: tapered tile widths TW (small first
tile shortens the fill, small last tile + interleaved per-jslice drain
shortens the tail), and the final add+store deferred to finish_stage
after the next tile's narrow smalls so W4d(d) completes ~2us earlier and
the next iteration's PE broadcast never stalls on it.

v7 design:
 - Grids fp16 on HOST (HBM 23.7MB/core); output fp16, host upconverts.
 - DVE merge in fp16 2x_1p mode (0.52 ns/col, HW-verified): products as
   4 chunk-paired ops [128,2,1536] (wq broadcast over the chunk axis via
   an explicit 0-stride AP dim) + 3 paired adds, halving op count.
 - PE cost is passes x cols x 0.85ns (observed throttled clock) + 93ns
   LDWEIGHTS per matmul.  v4 ran 13 column passes on PE; v7 runs 10:
   grids 0-2's C-chunks are pre-folded on DVE (t_i = g_c0 + g_c1, fp16
   2x), so logits need 5 accumulating matmuls per jslice instead of 8.
   Folding all 4 would tip DVE past PE; 3 balances the two engines.
 - Broadcast staging via [128,768] PSUM tiles: per grid per 768-block,
   two matmuls (N=512+256, each within one PSUM bank) + ONE wide ACT
   copy PSUM->SBUF fp16 (halves v4's ACT op count and sem load).
 - Narrow tail per jslice: exp (ACT, scale=1/C exp-trick, bias=b_proj),
   S4 denominator (PE, ones lhsT), reciprocal_approx_fast (DVE, fp32),
   W4 = E*R4 -> fp16 into a d-tile-wide W4d (next iter's bcast rhs).
 - Per-iter emission (engine queue order is what matters):
     dma_in(d+1) | DVE folds(d) | PE bcast(d-1) + ACT staging | DVE
     products/adds(d-1) + dma_out(d-1) | narrow(d) | (DVE recip/W4 last)
   Folds go first on DVE so PE's logits(d) unblock early; products(d-1)
   keep DVE busy while the narrow(d) PE->ACT->PE chain round-trips.
 - The flush (last tile) runs broadcast+wide in two 768-col halves so the
   DVE wide work of half 1 overlaps the PE/ACT broadcast of half 2
   (-7us of serial drain).
 - Negative results (measured): all-gpsimd broadcast (fp32 partition_
   broadcast) 182-186us from SBUF-port DVE stalls; gpsimd partition_all_
   reduce for the softmax denominator 191us (3.5-3.9us/op + ~1us sem
   events on the Q7); fold-4 180us; 1024/512 bcast staging blocks 161us.
"""

import os
import sys

import numpy as np

try:
    import concourse.bass as bass
except ImportError:  # fresh grading dir: concourse lives in the container repo
    sys.path.insert(0, "/opt/trn_rl_repo")
    import concourse.bass as bass

from contextlib import ExitStack

import concourse.tile as tile
from concourse import bacc, mybir
from concourse.bass_utils import run_bass_kernel_spmd

B, C, H, W = 8, 256, 96, 96
HW = H * W  # 9216
NCORES = 8
CPB = C // 128  # 2 partition chunks per core
MAXW = 1536  # pool sizing; per-tile widths taper at both ends
# small first tile shortens the fill (dma+narrow before any wide work);
# small last tile shortens the drain (bcast+wide after the last narrow)
TW = [512, 1536, 1536, 1536, 1536, 1536, 1024]
assert sum(TW) == HW and all(w % 512 == 0 for w in TW)
ND = len(TW)
NFOLD = 3  # grids 0..NFOLD-1 chunk-folded on DVE for the logits pass
BLK = 768  # bcast staging block (2 matmuls + 1 ACT copy per grid)

F32 = mybir.dt.float32
F16 = mybir.dt.float16
AF = mybir.ActivationFunctionType

_CACHE = {}


def bcast2(ap, n):
    """AP broadcast over a new middle 'chunk' axis of size n (stride 0)."""
    return bass.AP(ap.tensor, ap.offset, [ap.ap[0], [0, n], ap.ap[-1]])


def build_program():
    nc = bacc.Bacc("TRN2", debug=False, num_devices=NCORES)

    gall_d = nc.dram_tensor("gall", [4, C, HW], F16, kind="ExternalInput").ap()
    # fp16 constants: cols 0-511 sel (one-hot bcast lhsT rows 0-3), cols
    # 512-527 ws (w[o,i] at col 512+4i+o, replicated down partitions),
    # cols 528-531 ones4x4
    ch_d = nc.dram_tensor("cblob16", [128, 532], F16, kind="ExternalInput").ap()
    # fp32 constants: col 0 rows 0-3 = b_proj
    cb_d = nc.dram_tensor("cblob", [128, 1], F32, kind="ExternalInput").ap()
    out = nc.dram_tensor("out", [C, HW], F16, kind="ExternalOutput").ap()
    # HBM bounce for the DMA-routed weight broadcast of grids 2,3: DMA
    # rejects stride-0 SBUF partition dims, but a flat DRAM source row can
    # be re-read 128x ([[0,128],[1,w]]), turning the broadcast into pure
    # DMA-engine work (no PE pass, no ACT staging).  Double-buffered by
    # d%2 (written end of narrow(d), read at iter d+1's bcast).
    wscr = nc.dram_tensor("wscr", [2, 2, MAXW], F16, kind="Internal").ap()

    with tile.TileContext(nc) as tc, ExitStack() as ctx:
        const = ctx.enter_context(tc.tile_pool(name="const", bufs=1))
        gin = ctx.enter_context(tc.tile_pool(name="gin", bufs=3))
        outp = ctx.enter_context(tc.tile_pool(name="outp", bufs=2))
        foldp = ctx.enter_context(tc.tile_pool(name="foldp", bufs=2))
        narrow = ctx.enter_context(tc.tile_pool(name="narrow", bufs=3))
        wqp = ctx.enter_context(tc.tile_pool(name="wqp", bufs=2))
        prod = ctx.enter_context(tc.tile_pool(name="prod", bufs=3))
        qpool = ctx.enter_context(tc.tile_pool(name="qpool", bufs=3))
        ps_nar = ctx.enter_context(tc.tile_pool(name="psnar", bufs=2, space="PSUM"))
        ps_wb = ctx.enter_context(tc.tile_pool(name="pswb", bufs=2, space="PSUM"))

        ch = const.tile([128, 532], F16)
        nc.sync.dma_start(out=ch[:], in_=ch_d)
        cb = const.tile([128, 1], F32)
        nc.sync.dma_start(out=cb[:], in_=cb_d)
        bv = cb[0:4, 0:1]

        def ws_i(i):  # [128, 4] logits lhsT for grid i
            return ch[:, 512 + 4 * i : 512 + 4 * i + 4]

        def sel_i(i):  # [4, 128] bcast lhsT for grid i
            return ch[0:4, 128 * i : 128 * (i + 1)]

        ones4 = ch[0:4, 528:532]

        # Warmup matmul: absorbs the const-blob DMA wait on the PE clock.
        warm = ps_nar.tile([128, 512], F32, tag="smx")
        nc.tensor.matmul(warm[0:4, 0:16], lhsT=ch[0:4, 0:4], rhs=ch[0:4, 0:16],
                         start=True, stop=True)

        def fold_stage(gat, w):
            """DVE chunk-fold for grids 0..NFOLD-1 (fp16 2x): the logits
            contraction over (grid, chunk) shrinks from 8 to 5 matmuls."""
            ts = []
            for i in range(NFOLD):
                t = foldp.tile([128, MAXW], F16, tag=f"t{i}")
                nc.vector.tensor_add(t[:, 0:w], gat[:, i, 0, 0:w],
                                     gat[:, i, 1, 0:w])
                ts.append(t)
            return ts

        def narrow_stage(d, gat, ts):
            """Softmax chain: logits (jslice pairs) -> exp -> S4 -> recip
            -> W4 (fp16, d-tile-wide for next iter's bcast rhs).  Matmul
            outputs at PSUM base partition 0 (ISA constraint)."""
            jslc = [(x0, 512) for x0 in range(0, TW[d], 512)]
            W4d = narrow.tile([4, MAXW], F16, tag="W4", bufs=2)
            for pair in [jslc[k : k + 2] for k in range(0, len(jslc), 2)]:
                Ls, Es = [], []
                for x0, n in pair:
                    L = ps_nar.tile([128, 512], F32, tag="smx")
                    Ls.append(L[0:4, 0:n])
                    rhss = [ts[i][:, x0 : x0 + n] for i in range(NFOLD)]
                    rhss += [
                        gat[:, i, c, x0 : x0 + n]
                        for i in range(NFOLD, 4)
                        for c in range(CPB)
                    ]
                    lhss = [ws_i(i) for i in range(NFOLD)] + [
                        ws_i(i) for i in range(NFOLD, 4) for _ in range(CPB)
                    ]
                    for k, (lh, rh) in enumerate(zip(lhss, rhss)):
                        nc.tensor.matmul(Ls[-1], lhsT=lh, rhs=rh,
                                         start=(k == 0),
                                         stop=(k == len(rhss) - 1))
                for pi, (x0, n) in enumerate(pair):
                    E = narrow.tile([4, 512], F16, tag="E")
                    nc.scalar.activation(E[0:4, 0:n], Ls[pi], AF.Exp,
                                         bias=bv, scale=1.0 / C)
                    Es.append(E[0:4, 0:n])
                S4s = []
                for pi, (x0, n) in enumerate(pair):
                    S4 = ps_nar.tile([4, 512], F32, tag="S4")
                    nc.tensor.matmul(S4[0:4, 0:n], lhsT=ones4, rhs=Es[pi],
                                     start=True, stop=True)
                    S4s.append(S4[0:4, 0:n])
                for pi, (x0, n) in enumerate(pair):
                    R4 = narrow.tile([4, 512], F32, tag="R4", bufs=2)
                    nc.vector.reciprocal_approx_fast(R4[0:4, 0:n], S4s[pi])
                    nc.vector.tensor_mul(W4d[0:4, x0 : x0 + n], Es[pi],
                                         R4[0:4, 0:n])
            nc.sync.dma_start(out=wscr[d % 2, :, 0 : TW[d]],
                              in_=W4d[2:4, 0 : TW[d]])
            return W4d

        def bcast_stage(prev):
            """PE broadcast of W4 rows to 128 partitions, staged to fp16
            SBUF by wide [128,768] ACT copies (each matmul writes within a
            single PSUM bank; the copy spans banks, reads are unrestricted)."""
            if prev is None:
                return None
            d, gat, ot, W4d = prev
            w = TW[d]
            wq = {}
            for i in range(4):
                wqt = wqp.tile([128, MAXW], F16, tag=f"wq{i}")
                wq[i] = wqt
            for i in (2, 3):
                row = wscr[d % 2, i - 2 : i - 1, 0:w]
                src = bass.AP(row.tensor, row.offset, [[0, 128], [1, w]])
                nc.sync.dma_start(out=wq[i][:, 0:w], in_=src)
            for b0 in range(0, w, BLK):
                bw = min(BLK, w - b0)
                for i in (0, 1):
                    Wb = ps_wb.tile([128, BLK], F32, tag="wb")
                    for s0 in range(0, bw, 512):
                        n = min(512, bw - s0)
                        nc.tensor.matmul(
                            Wb[:, s0 : s0 + n],
                            lhsT=sel_i(i),
                            rhs=W4d[0:4, b0 + s0 : b0 + s0 + n],
                            start=True, stop=True,
                        )
                    nc.scalar.copy(wq[i][:, b0 : b0 + bw], Wb[:, 0:bw])
            return (d, gat, ot, wq)

        def wide_stage(staged):
            """DVE products + first add-tree level (chunk-paired ops).
            The final add + store happen in finish_stage AFTER the next
            tile's narrow smalls, so W4d(d) completes ~2us earlier and the
            next iteration's PE broadcast doesn't stall on it."""
            if staged is None:
                return None
            d, gat, ot, wq = staged
            w = TW[d]
            p = {}
            for i in range(4):
                pt = prod.tile([128, CPB, MAXW], F16, tag="p")
                nc.vector.tensor_mul(pt[:, :, 0:w], gat[:, i, :, 0:w],
                                     bcast2(wq[i][:, 0:w], CPB))
                p[i] = pt
                if i == 1:
                    q01 = qpool.tile([128, CPB, MAXW], F16, tag="q")
                    nc.vector.tensor_add(q01[:, :, 0:w], p[0][:, :, 0:w],
                                         p[1][:, :, 0:w])
            q23 = qpool.tile([128, CPB, MAXW], F16, tag="q")
            nc.vector.tensor_add(q23[:, :, 0:w], p[2][:, :, 0:w],
                                 p[3][:, :, 0:w])
            return (d, ot, q01, q23)

        def finish_stage(pend):
            if pend is None:
                return
            d, ot, q01, q23 = pend
            w = TW[d]
            nc.vector.tensor_add(ot[:, :, 0:w], q01[:, :, 0:w],
                                 q23[:, :, 0:w])
            n0 = sum(TW[:d])
            nc.sync.dma_start(
                out=out[:, n0 : n0 + w].rearrange("(c p) n -> p c n", c=CPB),
                in_=ot[:, :, 0:w],
            )

        def dma_in(d):
            n0 = sum(TW[:d])
            w = TW[d]
            gat = gin.tile([128, 4, CPB, MAXW], F16, tag="gall")
            nc.sync.dma_start(
                out=gat[:, :, :, 0:w],
                in_=gall_d[:, :, n0 : n0 + w].rearrange(
                    "i (c p) n -> p i c n", c=CPB
                ),
            )
            return gat

        gats = {0: dma_in(0)}
        prev = None  # (d, gat, ot, W4d) awaiting bcast+wide
        for d in range(ND - 1):
            gat = gats.pop(d)
            ts = fold_stage(gat, TW[d])
            # bcast first: its (tiny) broadcast DMAs must hit the sync queue
            # ahead of the 4.7MB dma_in so wq2/wq3 aren't delayed behind it
            staged = bcast_stage(prev)
            if d + 1 < ND:
                gats[d + 1] = dma_in(d + 1)
            pend = wide_stage(staged)
            ot = outp.tile([128, CPB, MAXW], F16, tag="ot")
            W4d = narrow_stage(d, gat, ts)
            finish_stage(pend)
            prev = (d, gat, ot, W4d)
        # Last tile: single-jslice narrow chains with the drain interleaved
        # per slice.  Each slice's recip/W4 feeds its drain broadcast
        # immediately, so the PE/ACT drain work overlaps the other slice's
        # DVE work instead of serializing after the whole narrow chain.
        d = ND - 1
        gat = gats.pop(d)
        ts = fold_stage(gat, TW[d])
        staged = bcast_stage(prev)
        finish_stage(wide_stage(staged))
        ot = outp.tile([128, CPB, MAXW], F16, tag="ot")
        w = TW[d]
        jslc = [(x0, 512) for x0 in range(0, w, 512)]
        W4d = narrow.tile([4, MAXW], F16, tag="W4", bufs=2)
        chain = []
        for x0, n in jslc:
            L = ps_nar.tile([128, 512], F32, tag="smx")
            Lv = L[0:4, 0:n]
            rhss = [ts[i][:, x0 : x0 + n] for i in range(NFOLD)] + [
                gat[:, i, c, x0 : x0 + n]
                for i in range(NFOLD, 4)
                for c in range(CPB)
            ]
            lhss = [ws_i(i) for i in range(NFOLD)] + [
                ws_i(i) for i in range(NFOLD, 4) for _ in range(CPB)
            ]
            for k, (lh, rh) in enumerate(zip(lhss, rhss)):
                nc.tensor.matmul(Lv, lhsT=lh, rhs=rh, start=(k == 0),
                                 stop=(k == len(rhss) - 1))
            E = narrow.tile([4, 512], F16, tag="E")
            nc.scalar.activation(E[0:4, 0:n], Lv, AF.Exp, bias=bv,
                                 scale=1.0 / C)
            S4 = ps_nar.tile([4, 512], F32, tag="S4")
            nc.tensor.matmul(S4[0:4, 0:n], lhsT=ones4, rhs=E[0:4, 0:n],
                             start=True, stop=True)
            chain.append((x0, n, E, S4))
        for x0, n, E, S4 in chain:
            R4 = narrow.tile([4, 512], F32, tag="R4", bufs=2)
            nc.vector.reciprocal_approx_fast(R4[0:4, 0:n], S4[0:4, 0:n])
            nc.vector.tensor_mul(W4d[0:4, x0 : x0 + n], E[0:4, 0:n],
                                 R4[0:4, 0:n])
        dr = []
        for x0, n, E, S4 in chain:
            wqh = {}
            for i in range(4):
                wqt = wqp.tile([128, MAXW], F16, tag=f"wq{i}")
                wqh[i] = wqt
                Wb = ps_wb.tile([128, BLK], F32, tag="wb")
                nc.tensor.matmul(Wb[:, 0:n], lhsT=sel_i(i),
                                 rhs=W4d[0:4, x0 : x0 + n],
                                 start=True, stop=True)
                nc.scalar.copy(wqt[:, x0 : x0 + n], Wb[:, 0:n])
            dr.append((x0, n, wqh))
        for x0, n, wqh in dr:
            p = {}
            for i in range(4):
                pt = prod.tile([128, CPB, MAXW], F16, tag="p")
                nc.vector.tensor_mul(
                    pt[:, :, 0:n], gat[:, i, :, x0 : x0 + n],
                    bcast2(wqh[i][:, x0 : x0 + n], CPB),
                )
                p[i] = pt
                if i == 1:
                    q01 = qpool.tile([128, CPB, MAXW], F16, tag="q")
                    nc.vector.tensor_add(q01[:, :, 0:n], p[0][:, :, 0:n],
                                         p[1][:, :, 0:n])
            q23 = qpool.tile([128, CPB, MAXW], F16, tag="q")
            nc.vector.tensor_add(q23[:, :, 0:n], p[2][:, :, 0:n],
                                 p[3][:, :, 0:n])
            nc.vector.tensor_add(ot[:, :, x0 : x0 + n], q01[:, :, 0:n],
                                 q23[:, :, 0:n])
            n0 = sum(TW[:d]) + x0
            nc.sync.dma_start(
                out=out[:, n0 : n0 + n].rearrange("(c p) n -> p c n", c=CPB),
                in_=ot[:, :, x0 : x0 + n],
            )

    nc.compile()
    return nc


def _get_program():
    if "nc" not in _CACHE:
        _CACHE["nc"] = build_program()
    return _CACHE["nc"]


def make_cblobs(w_proj, b_proj):
    w = np.asarray(w_proj, dtype=np.float32)
    b = np.asarray(b_proj, dtype=np.float32)
    ch = np.zeros((128, 532), dtype=np.float16)
    sel = np.repeat(np.eye(4, dtype=np.float16), 128, axis=1)
    ch[0:4, 0:512] = sel
    for i in range(4):
        for o in range(4):
            ch[:, 512 + 4 * i + o] = np.float16(w[o, i])
    ch[0:4, 528:532] = 1.0
    cb = np.zeros((128, 1), dtype=np.float32)
    cb[0:4, 0] = b
    return ch, cb


LAST_RESULT = None


def kernel(g0, g1, g2, g3, w_proj, b_proj):
    global LAST_RESULT
    nc = _get_program()

    ch, cb = make_cblobs(w_proj, b_proj)

    gall = np.stack(
        [np.asarray(x).reshape(B, C, HW).astype(np.float16) for x in (g0, g1, g2, g3)],
        axis=1,
    )  # (B, 4, C, HW) fp16
    in_maps = []
    for bi in range(NCORES):
        m = {"gall": np.ascontiguousarray(gall[bi]), "cblob16": ch, "cblob": cb}
        in_maps.append(m)

    res = run_bass_kernel_spmd(
        nc,
        in_maps,
        list(range(NCORES)),
        trace=bool(int(os.environ.get("CM_TRACE", "0"))),
        tmpdir=os.environ.get("CM_TRACE_DIR") or None,
    )
    LAST_RESULT = res
    out_full = np.stack(
        [
            res.results[bi]["out"].astype(np.float32).reshape(C, H, W)
            for bi in range(NCORES)
        ],
        axis=0,
    )
    return out_full
